# revision 23
# baseline (speedup 1.0000x reference)
"""Ernie4.5-VL MoE decoder layer on 8 TRN2 NeuronCores (Bass/Tile).

Strategy (expert-parallel, per the sharding hint):
 - Attention + routing replicated on all 8 cores (no collectives needed).
 - Each core owns 8 text + 8 vision experts (gate-weight columns permuted
   per-core so local experts are always columns 0..7 -> uniform SPMD NEFF).
 - Routed dispatch on-device: top-6 via DVE max8, compact ranks via PE
   prefix-matmul, slot table built with dma_scatter_add (zero-padded slots
   carry zero routing weight, so pads are harmless), token gather via
   dma_gather, combine via dma_scatter_add into the output accumulator.
 - Host only shards/permutes inputs, precomputes rope tables, and sums the
   8 partial outputs.
"""
import sys

if "/opt/trn_rl_repo" not in sys.path:
    sys.path.insert(0, "/opt/trn_rl_repo")

import numpy as np
import ml_dtypes

import concourse.bass as bass
import concourse.bacc as bacc
import concourse.mybir as mybir
from concourse.tile import TileContext
from concourse import bass_utils

F32 = mybir.dt.float32
BF16 = mybir.dt.bfloat16
I32 = mybir.dt.int32
I16 = mybir.dt.int16
AF = mybir.ActivationFunctionType
OP = mybir.AluOpType
AX = mybir.AxisListType

T, H = 2048, 1024
NH, NKV, D = 8, 2, 128
E, TOPK = 64, 6
IT, IV, IS = 512, 256, 1024
THETA = 500000.0
SECTIONS = (22, 22, 20)
EPS = 1e-5
NCORES = 8
EL = E // NCORES            # local experts per modality = 8
ISL = IS // NCORES          # local shared-inter slice = 128
CAP = 256                   # text per-expert token capacity (max load 231)
CAPV = 128                  # vision per-expert token capacity (max load 80)
TC = T // 128
SCALE = float(D) ** -0.5

_CACHE = {}


def _build():
    nc = bacc.Bacc("TRN2", target_bir_lowering=False, debug=False,
                   num_devices=NCORES)

    def din(name, shape, dt=F32):
        return nc.dram_tensor(name, shape, dt, kind="ExternalInput")

    io = dict(
        hid=din("hid", [T, H]),
        wqkv=din("wqkv", [H, (NH + NKV) * D], BF16),
        wv=din("wv", [H, NKV * D], BF16),
        wo=din("wo", [NH * D, H], BF16),
        cs2=din("cs2", [128, T], BF16),
        sn2=din("sn2", [128, T], BF16),
        gw=din("gw", [H, 2 * E]),
        bias2=din("bias2", [128, 2 * E]),
        masktv=din("masktv", [128, 2 * TC]),
        iota_f=din("iota_f", [128, TC]),
        iotacap=din("iotacap", [128, CAP]),
        ccap=din("ccap", [128, 16]),
        strictT=din("strictT", [128, 128]),
        diagmask=din("diagmask", [128, 128]),
        ones_1x128=din("ones_1x128", [1, 128]),
        ones_col=din("ones_col", [128, 1]),
        ones_cb=din("ones_cb", [128, 1], BF16),
        r16=din("r16", [16, 128]),
        idf32=din("idf32", [128, 128]),
        idbf=din("idbf", [128, 128], BF16),
        twg=din("twg", [EL, H, IT], BF16),
        twu=din("twu", [EL, H, IT], BF16),
        twd=din("twd", [EL, IT, H], BF16),
        vwg=din("vwg", [EL, H, IV], BF16),
        vwu=din("vwu", [EL, H, IV], BF16),
        vwd=din("vwd", [EL, IV, H], BF16),
        swg=din("swg", [H, ISL], BF16),
        swu=din("swu", [H, ISL], BF16),
        swd=din("swd", [ISL, H], BF16),
    )
    io["out_acc"] = nc.dram_tensor("out_acc", [T, H], F32, kind="ExternalOutput")

    with TileContext(nc) as tc:
        with tc.tile_pool(name="dram", bufs=1, space="DRAM") as dpool:
            io["x_bf"] = dpool.tile([T, H], BF16, name="x_bf")
            io["h_dram"] = dpool.tile([T, H], F32, name="h_dram")
            io["cmp_ids"] = dpool.tile([16, CAP], F32, name="cmp_ids")
            io["cmp_w"] = dpool.tile([16, CAP], F32, name="cmp_w")
            io["xT_dram"] = dpool.tile([8, 128, T], BF16, name="xT_dram")
            io["scr_dram"] = dpool.tile([T, 2 * E], F32, name="scr_dram")
            _trace_body(nc, tc, io)

    nc.compile()
    return nc


def _trace_body(nc, tc, io):
    hid = io["hid"]
    out_acc, x_bf, h_dram = io["out_acc"], io["x_bf"], io["h_dram"]
    cmp_ids, cmp_w = io["cmp_ids"], io["cmp_w"]
    xT_dram, scr_dram = io["xT_dram"], io["scr_dram"]
    scr_dram_r = scr_dram[:].rearrange("(m p) e -> m p e", p=128)
    hid_r = hid.ap().rearrange("(m p) h -> m p h", p=128)
    x_bf_r = x_bf[:].rearrange("(m p) h -> m p h", p=128)
    h_dram_r = h_dram[:].rearrange("(m p) h -> m p h", p=128)
    out_r = out_acc.ap().rearrange("(m p) h -> m p h", p=128)

    with tc.tile_pool(name="const", bufs=1) as cpool:
        def const(name, shape, dt=F32, src=None):
            t = cpool.tile(shape, dt, name="c_" + name)
            nc.sync.dma_start(t[:], (src if src is not None else io[name][:]))
            return t

        c_cs2 = const("cs2", [128, T], BF16)
        c_sn2 = const("sn2", [128, T], BF16)
        c_bias2 = const("bias2", [128, 2 * E])
        c_masktv = const("masktv", [128, 2 * TC])
        c_iota = const("iota_f", [128, TC])
        c_iotacap = const("iotacap", [128, CAP])
        c_ccap = const("ccap", [128, 16])
        c_strictT = const("strictT", [128, 128])
        c_diagmask = const("diagmask", [128, 128])
        c_ones_r = const("ones_1x128", [1, 128])
        c_ones_c = const("ones_col", [128, 1])
        c_ones_cb = const("ones_cb", [128, 1], BF16)
        c_r16 = const("r16", [16, 128])
        c_idf32 = const("idf32", [128, 128])
        c_idbf = const("idbf", [128, 128], BF16)
        c_gw = const("gw", [128, 8, 2 * E], F32,
                     io["gw"].ap().rearrange("(hc p) e -> p hc e", p=128))
        c_swg = const("swg", [128, 8, ISL], BF16,
                      io["swg"].ap().rearrange("(hc p) i -> p hc i", p=128))
        c_swu = const("swu", [128, 8, ISL], BF16,
                      io["swu"].ap().rearrange("(hc p) i -> p hc i", p=128))
        c_swd = const("swd", [128, H], BF16)

        with tc.tile_pool(name="bigpersist", bufs=1) as big:
            with tc.tile_pool(name="attnTp", bufs=1) as attnTp:
                attnT = attnTp.tile([128, 8, T], BF16, tag="attnT")

                with tc.tile_pool(name="xaTp", bufs=1) as xaTp:
                    xaT = xaTp.tile([128, 8, T], BF16, tag="xaT")

                    # ---- Phase A: rmsnorm(hid) -> xaT ----
                    with tc.tile_pool(name="pha", bufs=3) as pa, \
                         tc.tile_pool(name="pha_ps", bufs=3, space="PSUM") as pap:
                        ssq = pa.tile([128, TC], F32, tag="ssq")
                        for m in range(TC):
                            ht = pa.tile([128, H], F32, tag="hidt")
                            nc.sync.dma_start(ht[:], hid_r[m])
                            scr = pa.tile([128, H], F32, tag="sqscr")
                            nc.scalar.activation(scr[:], ht[:], AF.Square,
                                                 accum_out=ssq[:, m:m + 1])
                        rms = pa.tile([128, TC], F32, tag="rms")
                        nc.vector.tensor_scalar(out=rms[:], in0=ssq[:],
                                                scalar1=1.0 / H, scalar2=EPS,
                                                op0=OP.mult, op1=OP.add)
                        nc.scalar.activation(rms[:], rms[:], AF.Sqrt)
                        nc.vector.reciprocal(rms[:], rms[:])
                        for m in range(TC):
                            ht = pa.tile([128, H], F32, tag="hidt2")
                            nc.sync.dma_start(ht[:], hid_r[m])
                            xa = pa.tile([128, H], F32, tag="xarow")
                            nc.vector.tensor_scalar(out=xa[:], in0=ht[:],
                                                    scalar1=rms[:, m:m + 1],
                                                    scalar2=None, op0=OP.mult)
                            for hc in range(8):
                                pt = pap.tile([128, 128], F32, tag="tp")
                                nc.tensor.transpose(
                                    pt[:], xa[:, hc * 128:(hc + 1) * 128],
                                    c_idf32[:])
                                dst = xaT[:, hc, m * 128:(m + 1) * 128]
                                if hc % 2 == 0:
                                    nc.vector.tensor_copy(dst, pt[:])
                                else:
                                    nc.scalar.copy(dst, pt[:])

                    # ---- Phase B: qkv + rope + v ----
                    with tc.tile_pool(name="phb_kv", bufs=1) as pkv, \
                         tc.tile_pool(name="phb_w", bufs=2) as pbw, \
                         tc.tile_pool(name="phb", bufs=2) as pb, \
                         tc.tile_pool(name="phb_ps", bufs=3, space="PSUM") as pbp, \
                         tc.tile_pool(name="phb_pst", bufs=2, space="PSUM") as pbt:
                        kT = pkv.tile([128, NKV, T], BF16, tag="kT")
                        vrows = pkv.tile([128, TC, NKV * D], BF16, tag="vrows")
                        wv_sb = pkv.tile([128, 8, NKV * D], BF16, tag="wv_sb")
                        nc.sync.dma_start(
                            wv_sb[:],
                            io["wv"].ap().rearrange("(hc p) c -> p hc c", p=128))
                        wq_r = io["wqkv"].ap().rearrange("(hc p) c -> p hc c", p=128)

                        for grp in range(2):
                          qT = pb.tile([128, 4, T], BF16, tag="qT", bufs=1)
                          for mc in ([0, 1, 2, 3, 8, 9] if grp == 0 else [4, 5, 6, 7]):
                              wqm = pbw.tile([128, 8, 128], BF16, tag="wqm")
                              nc.sync.dma_start(
                                  wqm[:], wq_r[:, :, mc * 128:(mc + 1) * 128])
                              for n in range(4):
                                  sl = slice(n * 512, (n + 1) * 512)
                                  ps = pbp.tile([128, 512], F32, tag="qkps", bufs=2)
                                  for hc in range(8):
                                      nc.tensor.matmul(ps[:], wqm[:, hc, :],
                                                       xaT[:, hc, sl],
                                                       start=(hc == 0), stop=(hc == 7))
                                  qb = pb.tile([128, 512], BF16, tag="qb")
                                  nc.scalar.copy(qb[:], ps[:])
                                  qsw = pb.tile([128, 512], BF16, tag="qsw")
                                  nc.sync.dma_start(qsw[0:64, :], qb[64:128, :])
                                  nc.sync.dma_start(qsw[64:128, :], qb[0:64, :])
                                  nc.vector.tensor_tensor(out=qb[:], in0=qb[:],
                                                          in1=c_cs2[:, sl], op=OP.mult)
                                  nc.vector.tensor_tensor(out=qsw[:], in0=qsw[:],
                                                          in1=c_sn2[:, sl], op=OP.mult)
                                  dst = (qT[:, mc - 4 * grp, sl] if mc < NH
                                         else kT[:, mc - NH, sl])
                                  nc.vector.tensor_tensor(out=dst, in0=qb[:],
                                                          in1=qsw[:], op=OP.add)

                          if grp == 0:
                            for m in range(TC):
                              ps = pbp.tile([128, NKV * D], F32, tag="vps", bufs=1)
                              for hc in range(8):
                                  nc.tensor.matmul(
                                      ps[:], xaT[:, hc, m * 128:(m + 1) * 128],
                                      wv_sb[:, hc, :], start=(hc == 0), stop=(hc == 7))
                              nc.scalar.copy(vrows[:, m, :], ps[:])

                          # ---- Phase C: S_T scores / softmax / AV ----
                          # Scores computed transposed (k on partitions) so AV
                          # consumes them directly -- no per-tile P transposes.
                          # Denominator = ones-column matmul accumulated over
                          # k-tiles; per-head normalize happens once on AV out.
                          for h in range(4 * grp, 4 * grp + 4):
                              kv = h // (NH // NKV)
                              for qbk in range(16):
                                  nkc = qbk + 1
                                  qsl = slice(qbk * 128, qbk * 128 + 128)
                                  PTt = pb.tile([128, 16, 128], BF16, tag="PTt",
                                                bufs=2)
                                  dnp = pbt.tile([1, 128], F32, tag="dnps", bufs=1)
                                  for tkc in range(nkc):
                                      ps = pbt.tile([128, 128], F32, tag="stps",
                                                    bufs=2)
                                      nc.tensor.matmul(
                                          ps[:], kT[:, kv, tkc * 128:(tkc + 1) * 128],
                                          qT[:, h - 4 * grp, qsl],
                                          start=True, stop=True)
                                      if tkc == qbk:
                                          nc.vector.tensor_tensor(
                                              out=ps[:], in0=ps[:],
                                              in1=c_diagmask[:], op=OP.add)
                                      nc.scalar.activation(PTt[:, tkc, :], ps[:],
                                                           AF.Exp, scale=SCALE)
                                      nc.tensor.matmul(
                                          dnp[:], c_ones_cb[:], PTt[:, tkc, :],
                                          start=(tkc == 0), stop=(tkc == nkc - 1))
                                  av = pbt.tile([128, 128], F32, tag="avps", bufs=1)
                                  for tkc in range(nkc):
                                      nc.tensor.matmul(
                                          av[:], vrows[:, tkc, kv * D:(kv + 1) * D],
                                          PTt[:, tkc, :], start=(tkc == 0),
                                          stop=(tkc == nkc - 1))
                                  rdn = pb.tile([1, 128], F32, tag="rdn")
                                  nc.vector.reciprocal(rdn[:], dnp[:])
                                  bc = pbt.tile([128, 128], F32, tag="bcps", bufs=1)
                                  nc.tensor.matmul(bc[:], c_ones_r[:], rdn[:],
                                                   start=True, stop=True)
                                  bcs = pb.tile([128, 128], F32, tag="bcs")
                                  nc.scalar.copy(bcs[:], bc[:])
                                  nc.vector.tensor_tensor(
                                      out=attnT[:, h, qsl], in0=av[:], in1=bcs[:],
                                      op=OP.mult)

                # ---- O-proj, h rows, rms2, x rows/xT, gate logits ----
                with tc.tile_pool(name="pho", bufs=2) as po, \
                     tc.tile_pool(name="pho_ps", bufs=3, space="PSUM") as pop, \
                     tc.tile_pool(name="pho_pst", bufs=2, space="PSUM") as popt, \
                     tc.tile_pool(name="pho_psl", bufs=1, space="PSUM") as popl:
                    wo_sb = po.tile([128, 8, H], BF16, tag="wosb")
                    nc.sync.dma_start(
                        wo_sb[:],
                        io["wo"].ap().rearrange("(hc p) c -> p hc c", p=128))
                    ssq2 = po.tile([128, TC], F32, tag="ssq2")
                    for n in range(4):
                        sl = slice(n * 512, (n + 1) * 512)
                        oT = po.tile([128, 8, 512], F32, tag="oT")
                        for mh in range(8):
                            ps = pop.tile([128, 512], F32, tag="ops")
                            for kc in range(8):
                                nc.tensor.matmul(
                                    ps[:], wo_sb[:, kc, mh * 128:(mh + 1) * 128],
                                    attnT[:, kc, sl], start=(kc == 0),
                                    stop=(kc == 7))
                            nc.scalar.copy(oT[:, mh, :], ps[:])
                        for mi in range(4):
                            m = n * 4 + mi
                            ht = po.tile([128, H], F32, tag="hrein")
                            nc.sync.dma_start(ht[:], hid_r[m])
                            hrow = po.tile([128, H], F32, tag="hrow")
                            for hc in range(8):
                                pt = popt.tile([128, 128], F32, tag="otp")
                                nc.tensor.transpose(
                                    pt[:], oT[:, hc, mi * 128:(mi + 1) * 128],
                                    c_idf32[:])
                                nc.vector.scalar_tensor_tensor(
                                    out=hrow[:, hc * 128:(hc + 1) * 128],
                                    in0=ht[:, hc * 128:(hc + 1) * 128],
                                    scalar=1.0, in1=pt[:], op0=OP.mult, op1=OP.add)
                            nc.sync.dma_start(h_dram_r[m], hrow[:])
                            scr2 = po.tile([128, H], F32, tag="sqscr2")
                            nc.scalar.activation(scr2[:], hrow[:], AF.Square,
                                                 accum_out=ssq2[:, m:m + 1])
                    rms2 = po.tile([128, TC], F32, tag="rms2")
                    nc.vector.tensor_scalar(out=rms2[:], in0=ssq2[:],
                                            scalar1=1.0 / H, scalar2=EPS,
                                            op0=OP.mult, op1=OP.add)
                    nc.scalar.activation(rms2[:], rms2[:], AF.Sqrt)
                    nc.vector.reciprocal(rms2[:], rms2[:])
                    for m in range(TC):
                        hrow = po.tile([128, H], F32, tag="hrow2")
                        nc.sync.dma_start(hrow[:], h_dram_r[m])
                        xr = po.tile([128, H], F32, tag="xrow")
                        nc.vector.tensor_scalar(out=xr[:], in0=hrow[:],
                                                scalar1=rms2[:, m:m + 1],
                                                scalar2=None, op0=OP.mult)
                        xrb = po.tile([128, H], BF16, tag="xrowb")
                        nc.scalar.copy(xrb[:], xr[:])
                        nc.sync.dma_start(x_bf_r[m], xrb[:])
                        lg = popl.tile([128, 2 * E], F32, tag="lgps")
                        for hc in range(8):
                            pt = popt.tile([128, 128], F32, tag="xtp")
                            nc.tensor.transpose(pt[:],
                                                xr[:, hc * 128:(hc + 1) * 128],
                                                c_idf32[:])
                            xtf = po.tile([128, 128], F32, tag="xtf")
                            nc.vector.tensor_copy(xtf[:], pt[:])
                            xtb = po.tile([128, 128], BF16, tag="xtb")
                            nc.scalar.copy(xtb[:], xtf[:])
                            nc.sync.dma_start(
                                xT_dram[hc, :, m * 128:(m + 1) * 128], xtb[:])
                            nc.tensor.matmul(lg[:], xtf[:], c_gw[:, hc, :],
                                             start=(hc == 0), stop=(hc == 7))
                        scrm = po.tile([128, 2 * E], F32, tag="scrm")
                        nc.scalar.activation(scrm[:], lg[:], AF.Sigmoid)
                        nc.sync.dma_start(scr_dram_r[m], scrm[:])

            # ---- Phase D: routing + compact dispatch (matmul compaction) ----
            # For each local expert j, build R[t, s] = 1 iff token t has rank
            # s in expert j; PE-accumulate [ids+1; weight] @ R over t-blocks
            # into a compact [2, cap] slot table. Pads stay 0 -> id-1 = -1 ->
            # gather/scatter skip them (negative trailing indices ignored).
            with tc.tile_pool(name="phd", bufs=2) as pd, \
                 tc.tile_pool(name="phd_run", bufs=1) as pdr, \
                 tc.tile_pool(name="phd_ps", bufs=2, space="PSUM") as pdp, \
                 tc.tile_pool(name="phd_psr", bufs=2, space="PSUM") as pdpr, \
                 tc.tile_pool(name="phd_cmp", bufs=2, space="PSUM") as pdc:
                rkA = pdr.tile([128, TC, 16], F32, tag="rkA")
                isokA = pdr.tile([128, TC, 16], F32, tag="isokA")
                payI = pdr.tile([128, TC, 32], F32, tag="payI")
                runrow = pdr.tile([1, 16 * (TC + 1)], F32, tag="runrow")
                nc.vector.memset(runrow[:, 0:16], 0.0)
                for m in range(TC):
                    scrt = pd.tile([128, 2 * E], F32, tag="scrld")
                    nc.sync.dma_start(scrt[:], scr_dram_r[m])
                    scr = scrt[:]
                    sb2 = pd.tile([128, 2 * E], F32, tag="sb2")
                    nc.vector.tensor_tensor(out=sb2[:], in0=scr, in1=c_bias2[:],
                                            op=OP.add)
                    I16t = pd.tile([128, 16], F32, tag="I16")
                    W16 = pd.tile([128, 16], F32, tag="W16")
                    den = pd.tile([128, 2], F32, tag="den")
                    for br in range(2):
                        esl = slice(br * 64, br * 64 + 64)
                        m8 = pd.tile([128, 8], F32, tag="m8")
                        nc.vector.max(out=m8[:], in_=sb2[:, esl])
                        I0 = pd.tile([128, 64], F32, tag="I0")
                        nc.vector.tensor_scalar(
                            out=I0[:], in0=sb2[:, esl],
                            scalar1=m8[:, TOPK - 1:TOPK], scalar2=None,
                            op0=OP.is_ge)
                        selraw = pd.tile([128, 64], F32, tag="selraw")
                        nc.vector.scalar_tensor_tensor(
                            out=selraw[:], in0=I0[:], scalar=1.0,
                            in1=scr[:, esl], op0=OP.mult, op1=OP.mult,
                            accum_out=den[:, br:br + 1])
                        bm = c_masktv[:, br * TC + m:br * TC + m + 1]
                        nc.vector.tensor_tensor(
                            out=I16t[:, br * 8:br * 8 + 8], in0=I0[:, 0:8],
                            in1=bm.to_broadcast((128, 8)), op=OP.mult)
                        nc.vector.tensor_tensor(
                            out=W16[:, br * 8:br * 8 + 8], in0=selraw[:, 0:8],
                            in1=bm.to_broadcast((128, 8)), op=OP.mult)
                    rden = pd.tile([128, 2], F32, tag="rden")
                    nc.vector.reciprocal(rden[:], den[:])
                    for br in range(2):
                        nc.vector.tensor_scalar(
                            out=W16[:, br * 8:br * 8 + 8],
                            in0=W16[:, br * 8:br * 8 + 8],
                            scalar1=rden[:, br:br + 1], scalar2=None, op0=OP.mult)
                    rk = pdp.tile([128, 16], F32, tag="rkps")
                    nc.tensor.matmul(rk[:], c_strictT[:], I16t[:],
                                     start=True, stop=False)
                    nc.tensor.matmul(rk[:], c_ones_r[:],
                                     runrow[:, 16 * m:16 * m + 16],
                                     start=False, stop=True)
                    cs = pdpr.tile([1, 16], F32, tag="csps")
                    nc.tensor.matmul(cs[:], c_ones_c[:], I16t[:],
                                     start=True, stop=True)
                    cs_sb = pd.tile([1, 16], F32, tag="cssb")
                    nc.vector.tensor_copy(cs_sb[:], cs[:])
                    nc.vector.tensor_tensor(
                        out=runrow[:, 16 * (m + 1):16 * (m + 2)],
                        in0=runrow[:, 16 * m:16 * m + 16], in1=cs_sb[:], op=OP.add)
                    nc.vector.tensor_tensor(out=isokA[:, m, :], in0=rk[:],
                                            in1=c_ccap[:], op=OP.is_lt)
                    nc.vector.tensor_tensor(out=isokA[:, m, :],
                                            in0=isokA[:, m, :], in1=I16t[:],
                                            op=OP.mult)
                    nc.vector.tensor_copy(rkA[:, m, :], rk[:])
                    # payload lhs: interleaved [id+1, w] per expert
                    payv = payI[:, m, :].rearrange("p (g t) -> p g t", t=2)
                    nc.vector.tensor_copy(
                        payv[:, :, 0:1].rearrange("p g t -> p (g t)"),
                        c_iota[:, m:m + 1].to_broadcast((128, 16)))
                    nc.vector.tensor_copy(
                        payv[:, :, 1:2].rearrange("p g t -> p (g t)"), W16[:])
                for j in range(16):
                    capj = CAP if j < EL else CAPV
                    cmp_ps = pdc.tile([2, CAP], F32, tag="cmp_ps")
                    for m in range(TC):
                        R = pd.tile([128, CAP], F32, tag="Rsel")
                        nc.vector.tensor_scalar(
                            out=R[:, :capj], in0=c_iotacap[:, :capj],
                            scalar1=rkA[:, m, j:j + 1],
                            scalar2=isokA[:, m, j:j + 1],
                            op0=OP.is_equal, op1=OP.mult)
                        nc.tensor.matmul(cmp_ps[:, :capj],
                                         payI[:, m, 2 * j:2 * j + 2],
                                         R[:, :capj],
                                         start=(m == 0), stop=(m == TC - 1))
                    cmp_sb = pd.tile([2, CAP], F32, tag="cmp_sb")
                    nc.vector.tensor_copy(cmp_sb[:, :capj], cmp_ps[:, :capj])
                    nc.sync.dma_start(cmp_ids[j:j + 1, 0:capj],
                                      cmp_sb[0:1, 0:capj])
                    nc.sync.dma_start(cmp_w[j:j + 1, 0:capj],
                                      cmp_sb[1:2, 0:capj])

            # ---- Phase F: shared MLP + out_acc init ----
            with tc.tile_pool(name="phf", bufs=2) as pf, \
                 tc.tile_pool(name="phf_ps", bufs=2, space="PSUM") as pfp:
                guT = pf.tile([128, T], BF16, tag="sguT")
                for n in range(4):
                    sl = slice(n * 512, (n + 1) * 512)
                    xTn = pf.tile([128, 8, 512], BF16, tag="xTn")
                    nc.sync.dma_start(
                        xTn[:], xT_dram[:, :, sl].rearrange("hc p t -> p hc t"))
                    gp = pfp.tile([128, 512], F32, tag="sgps")
                    for hc in range(8):
                        nc.tensor.matmul(gp[:], c_swg[:, hc, :], xTn[:, hc, :],
                                         start=(hc == 0), stop=(hc == 7))
                    up = pfp.tile([128, 512], F32, tag="sups")
                    for hc in range(8):
                        nc.tensor.matmul(up[:], c_swu[:, hc, :], xTn[:, hc, :],
                                         start=(hc == 0), stop=(hc == 7))
                    gs = pf.tile([128, 512], F32, tag="sgs")
                    nc.scalar.activation(gs[:], gp[:], AF.Sigmoid)
                    nc.vector.tensor_tensor(out=gs[:], in0=gs[:], in1=gp[:],
                                            op=OP.mult)
                    nc.vector.tensor_tensor(out=guT[:, sl], in0=gs[:], in1=up[:],
                                            op=OP.mult)
                for m in range(TC):
                    hrow = pf.tile([128, H], F32, tag="hrow3")
                    nc.sync.dma_start(hrow[:], h_dram_r[m])
                    acc = pf.tile([128, H], F32, tag="accrow")
                    for n2 in range(2):
                        ps = pfp.tile([128, 512], F32, tag="sdps")
                        nc.tensor.matmul(ps[:], guT[:, m * 128:(m + 1) * 128],
                                         c_swd[:, n2 * 512:(n2 + 1) * 512],
                                         start=True, stop=True)
                        nc.vector.scalar_tensor_tensor(
                            out=acc[:, n2 * 512:(n2 + 1) * 512],
                            in0=hrow[:, n2 * 512:(n2 + 1) * 512],
                            scalar=1.0 / NCORES, in1=ps[:], op0=OP.mult, op1=OP.add)
                    nc.gpsimd.dma_start(out_r[m], acc[:])

            # ---- Phase E: experts ----
            with tc.tile_pool(name="phe_w", bufs=2) as pw, \
                 tc.tile_pool(name="phe", bufs=2) as pe, \
                 tc.tile_pool(name="phe_ps", bufs=2, space="PSUM") as pep, \
                 tc.tile_pool(name="phe_pst", bufs=2, space="PSUM") as pept:
                for j in range(2 * EL):
                    br, jj = (0, j) if j < EL else (1, j - EL)
                    II = IT if br == 0 else IV
                    capj = CAP if br == 0 else CAPV
                    ngr = capj // 128
                    wg_d, wu_d, wd_d = ((io["twg"], io["twu"], io["twd"]) if br == 0
                                        else (io["vwg"], io["vwu"], io["vwd"]))
                    nic = II // 128
                    wg_sb = pw.tile([128, 8, IT], BF16, tag="wg")
                    nc.sync.dma_start(
                        wg_sb[:, :, :II],
                        wg_d[jj].rearrange("(hc p) i -> p hc i", p=128))
                    wu_sb = pw.tile([128, 8, IT], BF16, tag="wu")
                    nc.sync.dma_start(
                        wu_sb[:, :, :II],
                        wu_d[jj].rearrange("(hc p) i -> p hc i", p=128))
                    wd_sb = pw.tile([128, IT // 128, H], BF16, tag="wd")
                    nc.sync.dma_start(
                        wd_sb[:, :nic, :],
                        wd_d[jj].rearrange("(ic p) hh -> p ic hh", p=128))
                    idxf = pe.tile([16, CAP // 16], F32, tag="idxf")
                    nc.sync.dma_start(
                        idxf[:, :capj // 16],
                        cmp_ids[j, 0:capj].rearrange("(c p) -> p c", p=16))
                    wcol = pe.tile([128, CAP // 128], F32, tag="wcol")
                    nc.sync.dma_start(
                        wcol[:, :ngr],
                        cmp_w[j, 0:capj].rearrange("(g p) -> p g", p=128))
                    rep = pept.tile([128, CAP // 16], F32, tag="reps", bufs=1)
                    nc.tensor.matmul(rep[:, :capj // 16], c_r16[:],
                                     idxf[:, :capj // 16], start=True, stop=True)
                    idx16 = pe.tile([128, CAP // 16], I16, tag="idx16")
                    nc.vector.tensor_copy(idx16[:, :capj // 16],
                                          rep[:, :capj // 16])
                    xg = pe.tile([128, CAP // 128, H], BF16, tag="xg")
                    nc.gpsimd.dma_gather(
                        out_ap=xg[:, :ngr, :], in_ap=x_bf[:],
                        idxs_ap=idx16[:, :capj // 16],
                        num_idxs=capj, num_idxs_reg=capj, elem_size=H)
                    xgf = pe.tile([128, CAP // 128, H], F32, tag="xgf")
                    nc.vector.tensor_copy(xgf[:, :ngr, :], xg[:, :ngr, :])
                    xgT = pe.tile([128, 8, CAP], BF16, tag="xgT")
                    for g in range(ngr):
                        for hc in range(8):
                            pt = pept.tile([128, 128], F32, tag="etp")
                            nc.tensor.transpose(
                                pt[:], xgf[:, g, hc * 128:(hc + 1) * 128], c_idf32[:])
                            dst = xgT[:, hc, g * 128:(g + 1) * 128]
                            if hc % 2 == 0:
                                nc.vector.tensor_copy(dst, pt[:])
                            else:
                                nc.scalar.copy(dst, pt[:])
                    eguT = pe.tile([128, IT // 128, CAP], BF16, tag="eguT")
                    for ic in range(nic):
                        isl = slice(ic * 128, (ic + 1) * 128)
                        gp = pep.tile([128, CAP], F32, tag="egps")
                        for hc in range(8):
                            nc.tensor.matmul(gp[:, :capj], wg_sb[:, hc, isl],
                                             xgT[:, hc, :capj], start=(hc == 0),
                                             stop=(hc == 7))
                        up = pep.tile([128, CAP], F32, tag="eups")
                        for hc in range(8):
                            nc.tensor.matmul(up[:, :capj], wu_sb[:, hc, isl],
                                             xgT[:, hc, :capj], start=(hc == 0),
                                             stop=(hc == 7))
                        gs = pe.tile([128, CAP], F32, tag="egs")
                        nc.scalar.activation(gs[:, :capj], gp[:, :capj], AF.Sigmoid)
                        nc.vector.tensor_tensor(out=gs[:, :capj], in0=gs[:, :capj],
                                                in1=gp[:, :capj], op=OP.mult)
                        nc.vector.tensor_tensor(out=eguT[:, ic, :capj],
                                                in0=gs[:, :capj],
                                                in1=up[:, :capj], op=OP.mult)
                    yrows = pe.tile([128, CAP // 128, H], F32, tag="yrows")
                    for g in range(ngr):
                        gsl = slice(g * 128, (g + 1) * 128)
                        for n2 in range(2):
                            ps = pep.tile([128, 512], F32, tag="edps", bufs=1)
                            for ic in range(nic):
                                nc.tensor.matmul(
                                    ps[:], eguT[:, ic, gsl],
                                    wd_sb[:, ic, n2 * 512:(n2 + 1) * 512],
                                    start=(ic == 0), stop=(ic == nic - 1))
                            nc.vector.tensor_scalar(
                                out=yrows[:, g, n2 * 512:(n2 + 1) * 512],
                                in0=ps[:], scalar1=wcol[:, g:g + 1], scalar2=None,
                                op0=OP.mult)
                    nc.gpsimd.dma_scatter_add(
                        out_ap=out_acc[:], in_ap=yrows[:, :ngr, :],
                        idxs_ap=idx16[:, :capj // 16],
                        num_idxs=capj, num_idxs_reg=capj, elem_size=H)


# ------------------------- host side -------------------------

def _rope_tables(positions):
    pos = positions.reshape(3, T).astype(np.float64)
    inv_freq = 1.0 / (THETA ** (np.arange(0, D, 2, dtype=np.float64) / D))
    axis_idx = np.repeat(np.arange(3), SECTIONS)
    theta = pos[axis_idx, :] * inv_freq[:, None]
    cos, sin = np.cos(theta), np.sin(theta)
    cs2 = np.concatenate([cos, cos], 0)
    sn2 = np.concatenate([-sin, sin], 0)
    return (cs2.astype(ml_dtypes.bfloat16), sn2.astype(ml_dtypes.bfloat16))


def _prep_core_inputs(inputs):
    f32 = np.float32
    bf = ml_dtypes.bfloat16
    hidden = np.asarray(inputs["hidden_states"], f32).reshape(T, H)
    positions = np.asarray(inputs["positions"]).reshape(3, T)
    vmask = np.asarray(inputs["visual_token_mask"]).reshape(T).astype(f32)
    w_qkv = np.asarray(inputs["w_qkv"], f32)
    w_o = np.asarray(inputs["w_o"], f32)
    ln1 = np.asarray(inputs["ln1_w"], f32)
    ln2 = np.asarray(inputs["ln2_w"], f32)
    tgw = np.asarray(inputs["text_gate_w"], f32)
    vgw = np.asarray(inputs["vision_gate_w"], f32)
    cbias = np.asarray(inputs["corr_bias"], f32)
    tw_g = np.asarray(inputs["tw_gate"], f32)
    tw_u = np.asarray(inputs["tw_up"], f32)
    tw_d = np.asarray(inputs["tw_down"], f32)
    vw_g = np.asarray(inputs["vw_gate"], f32)
    vw_u = np.asarray(inputs["vw_up"], f32)
    vw_d = np.asarray(inputs["vw_down"], f32)
    sw_g = np.asarray(inputs["sw_gate"], f32)
    sw_u = np.asarray(inputs["sw_up"], f32)
    sw_d = np.asarray(inputs["sw_down"], f32)

    wqkv_f = ln1[:, None] * w_qkv
    perm = np.concatenate([np.arange(0, D, 2), np.arange(1, D, 2)])
    qcols = np.concatenate(
        [wqkv_f[:, h * D:(h + 1) * D][:, perm] for h in range(NH)], 1)
    kbase = NH * D
    kcols = np.concatenate(
        [wqkv_f[:, kbase + h * D:kbase + (h + 1) * D][:, perm]
         for h in range(NKV)], 1)
    vbase = (NH + NKV) * D
    wqkv_in = np.concatenate([qcols, kcols], 1).astype(bf)
    wqkv_sw = wqkv_in.reshape(H, NH + NKV, 2, 64)[:, :, ::-1, :].reshape(
        H, (NH + NKV) * D).copy()
    wv_in = wqkv_f[:, vbase:vbase + NKV * D].astype(bf)
    wo_in = w_o.astype(bf)
    cs2, sn2 = _rope_tables(positions)

    strictT = np.triu(np.ones((128, 128), f32), 1)
    # S_T layout [k, q]: mask k > q = strictly-lower
    diagmask = (np.tril(np.ones((128, 128), f32), -1) * (-1e30)).astype(f32)
    ones_r = np.ones((1, 128), f32)
    ones_c = np.ones((128, 1), f32)
    ones_cb = np.ones((128, 1), bf)
    r16 = np.zeros((16, 128), f32)
    for p in range(128):
        r16[p % 16, p] = 1.0
    idf32 = np.eye(128, dtype=f32)
    idbf = np.eye(128, dtype=f32).astype(bf)
    iota_f = np.zeros((128, TC), f32)
    for m in range(TC):
        iota_f[:, m] = np.arange(128) + 128 * m   # pads land on row 0, w=0
    iotacap = np.broadcast_to(np.arange(CAP, dtype=f32), (128, CAP)).copy()
    masktv = np.zeros((128, 2 * TC), f32)
    for m in range(TC):
        masktv[:, m] = 1.0 - vmask[m * 128:(m + 1) * 128]
        masktv[:, TC + m] = vmask[m * 128:(m + 1) * 128]
    ccap = np.broadcast_to(
        np.array([CAP] * EL + [CAPV] * EL, f32), (128, 16)).copy()

    ln2c = ln2[:, None]
    in_maps = []
    for c in range(NCORES):
        eperm = np.concatenate([np.arange(c * EL, (c + 1) * EL),
                                np.arange(0, c * EL),
                                np.arange((c + 1) * EL, E)])
        gw_c = np.concatenate([(ln2c * tgw)[:, eperm],
                               (ln2c * vgw)[:, eperm]], 1).astype(f32)
        bias_c = np.concatenate([cbias[0][eperm], cbias[1][eperm]])
        bias2 = np.broadcast_to(bias_c, (128, 2 * E)).astype(f32).copy()
        loc = np.arange(c * EL, (c + 1) * EL)
        in_maps.append(dict(
            hid=hidden, wqkv=wqkv_in, wqkvS=wqkv_sw, wv=wv_in, wo=wo_in,
            cs2=cs2, sn2=sn2,
            gw=gw_c, bias2=bias2, masktv=masktv, iota_f=iota_f,
            iotacap=iotacap, ccap=ccap, strictT=strictT, diagmask=diagmask,
            ones_1x128=ones_r, ones_col=ones_c, ones_cb=ones_cb, r16=r16,
            idf32=idf32, idbf=idbf,
            twg=(ln2c * tw_g[loc]).astype(bf),
            twu=(ln2c * tw_u[loc]).astype(bf),
            twd=tw_d[loc].astype(bf),
            vwg=(ln2c * vw_g[loc]).astype(bf),
            vwu=(ln2c * vw_u[loc]).astype(bf),
            vwd=vw_d[loc].astype(bf),
            swg=(ln2c * sw_g[:, c * ISL:(c + 1) * ISL]).astype(bf),
            swu=(ln2c * sw_u[:, c * ISL:(c + 1) * ISL]).astype(bf),
            swd=sw_d[c * ISL:(c + 1) * ISL, :].astype(bf),
        ))
    return in_maps


def kernel(**inputs):
    if "nc" not in _CACHE:
        _CACHE["nc"] = _build()
    nc = _CACHE["nc"]
    in_maps = _prep_core_inputs(inputs)
    res = bass_utils.run_bass_kernel_spmd(nc, in_maps,
                                          core_ids=list(range(NCORES)))
    out = np.zeros((T, H), np.float64)
    for r in res.results:
        out += r["out_acc"].astype(np.float64)
    return out.astype(np.float32).reshape(1, T, H)


if __name__ == "__main__":
    nc = _build()
    print("build ok")

